# revision 9
# baseline (speedup 1.0000x reference)
"""Trainium2 Bass kernel for nn_Bilinear_70222715290053.

Problem: x [128, 224, 224, 5] f32 where channels 0:3 are an image and
channels 3,4 are per-pixel displacements (dx, dy). Output [128,224,224,3]:
  out[b,i,j,:] = img[b, int(mod(i+dy, 224)), int(mod(j+dx, 224)), :]

Key property: dx, dy ~ N(0,1), so |displacement| <= ~5.5 — the gather is a
LOCAL warp within a 13x13 window (verified on the fixed inputs; kernel()
checks the bound at runtime and falls back if violated).

Strategy (pure data parallel, batch sharded 8 ways — 16 images/core):
  - Rows live in the PARTITION dim: a round processes G=4 images x 112
    output rows on 124 partitions (112 rows + 6 halo rows each side, with
    mod-224 wrap rows loaded into the halo slots). A window shift (oy, ox)
    is then just a partition-offset + free-offset AP — no data duplication.
  - Planarize the 3 image channels into x-wrap-padded planes [124, G*236].
  - Compute source indices exactly in f32 (bit-exact vs the CPU jax
    reference: wrap via compare+add/sub, floor via round-nearest convert +
    is_gt fixup, clamp at 223), normalize to window offsets, fold into a
    single code = offy*13 + offx (fp16).
  - 169 select terms: mask = is_equal(code, t) [fp16, DVE 2x mode], then 3
    copy_predicated moves from the (oy, ox)-shifted plane APs into the
    interleaved output tile. Every pixel matches exactly one term.
  - DMA the interleaved [112, G*672] tile to the output.

Self-contained: builds the Bass module, compiles through neuronx_cc via the
bass2jax custom call, and runs SPMD on 8 NeuronCores via shard_map.
"""

import sys

sys.path.insert(0, "/opt/trn_rl_repo")

import numpy as np

_CACHE = {}

_B, _H, _W = 16, 224, 224  # per-core shard
_HALO = 6
_G = 4  # images per round


def _build_module(B=_B, H=_H, W=_W, G=_G, HALO=_HALO):
    from concourse import mybir, bacc
    import concourse.tile as tile

    F32 = mybir.dt.float32
    F16 = mybir.dt.float16
    I32 = mybir.dt.int32
    Alu = mybir.AluOpType

    NB = 2               # row blocks per image
    RB = H // NB         # 112 output rows per block
    WPAD = W + 2 * HALO  # 236 x-padded plane width
    R5 = W * 5
    R3 = W * 3
    # Band partition layout: 0..RB-1 = central rows r0..r0+RB-1,
    # RB..RB+HALO-1 = top halo rows r0-HALO..r0-1 (mod H),
    # RB+HALO..RB+2*HALO-1 = bottom halo rows r0+RB..r0+RB+HALO-1 (mod H).
    PT = RB            # top halo partition base (112)
    PB = RB + HALO     # bottom halo partition base (118)
    BAND = RB + 2 * HALO

    nc = bacc.Bacc(None, target_bir_lowering=False)
    x = nc.declare_dram_parameter("x", [B * H, R5], F32, isOutput=False)
    y = nc.declare_dram_parameter("y", [B * H, R3], F32, isOutput=True)

    with tile.TileContext(nc) as tc:
        with (
            tc.tile_pool(name="consts", bufs=1) as cpool,
            tc.tile_pool(name="rec", bufs=2) as rpool,
            tc.tile_pool(name="planes", bufs=2) as ppool,
            tc.tile_pool(name="shift", bufs=2) as spool,
            tc.tile_pool(name="tmp", bufs=1) as tpool,
            tc.tile_pool(name="outp", bufs=2) as opool,
        ):
            jpat = cpool.tile([128, G * W], F32, tag="jpat")
            nc.gpsimd.iota(
                jpat[:], pattern=[[0, G], [1, W]], base=0, channel_multiplier=0,
                allow_small_or_imprecise_dtypes=True,
            )
            rowq = cpool.tile([128, 1], F32, tag="rowq")
            nc.gpsimd.iota(
                rowq[:], pattern=[[0, 1]], base=0, channel_multiplier=1,
                allow_small_or_imprecise_dtypes=True,
            )

            for g0 in range(0, B, G):
                for blk in range(NB):
                    r0 = blk * RB
                    rtop = (r0 - HALO) % H    # first top-halo row
                    rbot = (r0 + RB) % H      # first bottom-halo row
                    rec = rpool.tile([128, G * R5], F32, tag="rec")
                    for gi in range(G):
                        b = g0 + gi
                        fs = slice(gi * R5, (gi + 1) * R5)
                        nc.sync.dma_start(
                            out=rec[0:RB, fs],
                            in_=x[b * H + r0 : b * H + r0 + RB, :],
                        )
                        nc.sync.dma_start(
                            out=rec[PT : PT + HALO, fs],
                            in_=x[b * H + rtop : b * H + rtop + HALO, :],
                        )
                        nc.sync.dma_start(
                            out=rec[PB : PB + HALO, fs],
                            in_=x[b * H + rbot : b * H + rbot + HALO, :],
                        )

                    rec4 = rec[:].rearrange("p (g w c) -> p g w c", g=G, c=5)

                    # x-wrap-padded channel planes [BAND, G, WPAD] (band order)
                    planes = []
                    for c in range(3):
                        pc = ppool.tile([128, G * WPAD], F32, tag=f"plane{c}")
                        pc3 = pc[:].rearrange("p (g w) -> p g w", g=G)
                        nc.vector.tensor_copy(
                            out=pc3[0:BAND, :, HALO : HALO + W],
                            in_=rec4[0:BAND, :, :, c : c + 1].rearrange(
                                "p g w k -> p g (w k)"
                            ),
                        )
                        nc.vector.tensor_copy(
                            out=pc3[0:BAND, :, 0:HALO],
                            in_=rec4[0:BAND, :, W - HALO : W, c : c + 1].rearrange(
                                "p g w k -> p g (w k)"
                            ),
                        )
                        nc.vector.tensor_copy(
                            out=pc3[0:BAND, :, HALO + W : WPAD],
                            in_=rec4[0:BAND, :, 0:HALO, c : c + 1].rearrange(
                                "p g w k -> p g (w k)"
                            ),
                        )
                        planes.append(pc)

                    dxv = rec4[0:RB, :, :, 3:4].rearrange("p g w k -> p g (w k)")
                    dyv = rec4[0:RB, :, :, 4:5].rearrange("p g w k -> p g (w k)")

                    A = tpool.tile([128, G * W], F32, tag="A")
                    M = tpool.tile([128, G * W], F32, tag="M")
                    XI = tpool.tile([128, G * W], I32, tag="XI")
                    OX = tpool.tile([128, G * W], F32, tag="OX")
                    OY = tpool.tile([128, G * W], F32, tag="OY")

                    def s(t):
                        return t[0:RB, :]

                    def s3(t):
                        return t[0:RB, :].rearrange("p (g w) -> p g w", g=G)

                    def floor_clamp_off(t, out_off, pat_is_j):
                        """out_off = wrapnorm(min(floor(t), 223) - base)."""
                        # floor via round-nearest int convert + is_gt fixup
                        nc.vector.tensor_copy(out=s(XI), in_=t)
                        nc.vector.tensor_copy(out=out_off, in_=s(XI))
                        nc.vector.tensor_tensor(
                            out=s(M), in0=out_off, in1=t, op=Alu.is_gt
                        )
                        nc.vector.tensor_tensor(
                            out=out_off, in0=out_off, in1=s(M), op=Alu.subtract
                        )
                        nc.vector.tensor_scalar(
                            out=out_off, in0=out_off,
                            scalar1=float(W - 1), scalar2=0.0,
                            op0=Alu.min, op1=Alu.max,
                        )
                        # subtract base index (j or i)
                        if pat_is_j:
                            nc.vector.tensor_tensor(
                                out=out_off, in0=out_off,
                                in1=jpat[0:RB, :],
                                op=Alu.subtract,
                            )
                        else:
                            nc.vector.tensor_scalar(
                                out=out_off, in0=out_off,
                                scalar1=rowq[0:RB, 0:1], scalar2=float(r0),
                                op0=Alu.subtract, op1=Alu.subtract,
                            )
                        # normalize wrap offsets into [-HALO, HALO]
                        nc.vector.tensor_scalar(
                            out=s(M), in0=out_off, scalar1=112.0, scalar2=None,
                            op0=Alu.is_gt,
                        )
                        nc.vector.scalar_tensor_tensor(
                            out=out_off, in0=s(M), scalar=-224.0, in1=out_off,
                            op0=Alu.mult, op1=Alu.add,
                        )
                        nc.vector.tensor_scalar(
                            out=s(M), in0=out_off, scalar1=-112.0, scalar2=None,
                            op0=Alu.is_lt,
                        )
                        nc.vector.scalar_tensor_tensor(
                            out=out_off, in0=s(M), scalar=224.0, in1=out_off,
                            op0=Alu.mult, op1=Alu.add,
                        )

                    # X: tX = j + dx, wrap into [0, 224], floor, clamp, - j
                    nc.vector.tensor_tensor(
                        out=s(A), in0=dxv, in1=jpat[0:RB, :], op=Alu.add,
                    )
                    nc.vector.tensor_scalar(
                        out=s(M), in0=s(A), scalar1=float(W), scalar2=None,
                        op0=Alu.is_ge,
                    )
                    nc.vector.scalar_tensor_tensor(
                        out=s(A), in0=s(M), scalar=float(-W), in1=s(A),
                        op0=Alu.mult, op1=Alu.add,
                    )
                    nc.vector.tensor_scalar(
                        out=s(M), in0=s(A), scalar1=0.0, scalar2=None,
                        op0=Alu.is_lt,
                    )
                    nc.vector.scalar_tensor_tensor(
                        out=s(A), in0=s(M), scalar=float(W), in1=s(A),
                        op0=Alu.mult, op1=Alu.add,
                    )
                    floor_clamp_off(s(A), s(OX), pat_is_j=True)

                    # Y: tY = (i=r0+q) + dy, wrap, floor, clamp, - i
                    nc.vector.tensor_scalar(
                        out=s(A), in0=dyv, scalar1=rowq[0:RB, 0:1],
                        scalar2=float(r0), op0=Alu.add, op1=Alu.add,
                    )
                    if blk == NB - 1:
                        nc.vector.tensor_scalar(
                            out=s(M), in0=s(A), scalar1=float(H), scalar2=None,
                            op0=Alu.is_ge,
                        )
                        nc.vector.scalar_tensor_tensor(
                            out=s(A), in0=s(M), scalar=float(-H), in1=s(A),
                            op0=Alu.mult, op1=Alu.add,
                        )
                    if blk == 0:
                        nc.vector.tensor_scalar(
                            out=s(M), in0=s(A), scalar1=0.0, scalar2=None,
                            op0=Alu.is_lt,
                        )
                        nc.vector.scalar_tensor_tensor(
                            out=s(A), in0=s(M), scalar=float(H), in1=s(A),
                            op0=Alu.mult, op1=Alu.add,
                        )
                    floor_clamp_off(s(A), s(OY), pat_is_j=False)

                    # code = offy*13 + offx in [-84, 84]
                    nc.vector.scalar_tensor_tensor(
                        out=s(A), in0=s(OY), scalar=13.0, in1=s(OX),
                        op0=Alu.mult, op1=Alu.add,
                    )
                    codeh = tpool.tile([128, G * W], F16, tag="codeh")
                    nc.vector.tensor_copy(out=s(codeh), in_=s(A))

                    # Pad per-image strides of mask/out so their interp views
                    # cannot dim-merge (copy_predicated needs all three
                    # operand views shaped identically (RB, G, W)).
                    OSTR = R3 + 16  # 688
                    MSTR = W + 16   # 240
                    out_t = opool.tile([128, G * OSTR], F32, tag="out")
                    out4 = (
                        out_t[:]
                        .rearrange("p (g q) -> p g q", g=G)[:, :, 0:R3]
                        .rearrange("p g (w c) -> p g w c", c=3)
                    )
                    maskh = tpool.tile([128, G * MSTR], F16, tag="maskh")
                    mview = maskh[:].rearrange("p (g q) -> p g q", g=G)[
                        0:RB, :, 0:W
                    ]

                    for oy in range(-HALO, HALO + 1):
                        # partition-shifted copies (DMA may start at any
                        # partition; compute engines may not). Band layout
                        # makes each shift at most 2 contiguous pieces.
                        if oy == 0:
                            sps = planes
                        else:
                            sps = []
                            for c in range(3):
                                sp = spool.tile(
                                    [128, G * WPAD], F32, tag=f"sp{c}"
                                )
                                if oy < 0:
                                    k = -oy
                                    nc.sync.dma_start(
                                        out=sp[0:k, :],
                                        in_=planes[c][PB - k : PB, :],
                                    )
                                    nc.sync.dma_start(
                                        out=sp[k:RB, :],
                                        in_=planes[c][0 : RB - k, :],
                                    )
                                else:
                                    nc.sync.dma_start(
                                        out=sp[0 : RB - oy, :],
                                        in_=planes[c][oy:RB, :],
                                    )
                                    nc.sync.dma_start(
                                        out=sp[RB - oy : RB, :],
                                        in_=planes[c][PB : PB + oy, :],
                                    )
                                sps.append(sp)
                        spv = [
                            t[:].rearrange("p (g w) -> p g w", g=G) for t in sps
                        ]
                        for ox in range(-HALO, HALO + 1):
                            nc.vector.tensor_scalar(
                                out=mview, in0=s3(codeh),
                                scalar1=float(oy * 13 + ox), scalar2=None,
                                op0=Alu.is_equal,
                            )
                            for c in range(3):
                                nc.vector.copy_predicated(
                                    out4[0:RB, :, :, c : c + 1].rearrange(
                                        "p g w k -> p g (w k)"
                                    ),
                                    mview,
                                    spv[c][0:RB, :, HALO + ox : HALO + ox + W],
                                )

                    for gi in range(G):
                        b = g0 + gi
                        nc.sync.dma_start(
                            out=y[b * H + r0 : b * H + r0 + RB, :],
                            in_=out_t[0:RB, gi * OSTR : gi * OSTR + R3],
                        )
    return nc


def _split_multiwait_drains(nc):
    """This walrus build accepts one sync wait per Drain (TPB_CTRL); split
    the Tile epilogue's multi-wait drains into single-wait chains."""
    import copy
    import bass_rust
    from concourse import mybir

    changed = False
    new_functions = []
    for function in nc.m.functions:
        new_function = copy.replace(function, blocks=[])
        new_function.set_allocations_from_list(function.allocations)
        for block in function.blocks:
            new_insts = []
            for ins in block.instructions:
                si = ins.sync_info
                if (
                    isinstance(ins, (mybir.InstDrain, mybir.InstNoOp))
                    and si is not None
                    and len(si.on_wait) > 1
                ):
                    changed = True
                    waits = list(si.on_wait)
                    for i, w in enumerate(waits[:-1]):
                        d = mybir.InstDrain(
                            name=f"{ins.name}_sw{i}", ins=[], outs=[],
                            bass_is_fusable=False,
                        )
                        d.engine = ins.engine
                        d.sync_info = bass_rust.SyncInfo(on_wait=[w], on_update=[])
                        new_insts.append(d)
                    ins.sync_info = bass_rust.SyncInfo(
                        on_wait=[waits[-1]], on_update=list(si.on_update)
                    )
                new_insts.append(ins)
            new_function.blocks.append(copy.replace(block, instructions=new_insts))
        new_functions.append(new_function)
    if changed:
        nc.m = copy.replace(nc.m, functions=new_functions)
    return nc


class _Runner:
    def __init__(self, nc, n_cores=8):
        import jax
        from jax.sharding import Mesh, PartitionSpec, NamedSharding
        from jax.experimental.shard_map import shard_map
        from concourse import mybir
        from concourse.bass2jax import (
            _bass_exec_p,
            install_neuronx_cc_hook,
            partition_id_tensor,
        )

        install_neuronx_cc_hook()
        if not nc.is_finalized():
            nc.finalize()
        _split_multiwait_drains(nc)

        self.jax = jax
        partition_name = (
            nc.partition_id_tensor.name if nc.partition_id_tensor else None
        )
        in_names, out_names, out_avals, zero_shapes = [], [], [], []
        for alloc in nc.m.functions[0].allocations:
            if not isinstance(alloc, mybir.MemoryLocationSet):
                continue
            name = alloc.memorylocations[0].name
            if alloc.kind == "ExternalInput":
                if name != partition_name:
                    in_names.append(name)
            elif alloc.kind == "ExternalOutput":
                out_names.append(name)
                shape = tuple(alloc.tensor_shape)
                dtype = mybir.dt.np(alloc.dtype)
                out_avals.append(jax.core.ShapedArray(shape, dtype))
                zero_shapes.append((shape, dtype))
        n_params = len(in_names)
        n_outs = len(out_avals)
        all_in_names = list(in_names) + list(out_names)
        if partition_name is not None:
            all_in_names.append(partition_name)
        donate = tuple(range(n_params, n_params + n_outs))

        def _body(*args):
            operands = list(args)
            if partition_name is not None:
                operands.append(partition_id_tensor())
            outs = _bass_exec_p.bind(
                *operands,
                out_avals=tuple(out_avals),
                in_names=tuple(all_in_names),
                out_names=tuple(out_names),
                lowering_input_output_aliases=(),
                sim_require_finite=True,
                sim_require_nnan=True,
                nc=nc,
            )
            return tuple(outs)

        devices = jax.devices()[:n_cores]
        mesh = Mesh(np.asarray(devices), ("core",))
        in_specs = (PartitionSpec("core"),) * (n_params + n_outs)
        out_specs = (PartitionSpec("core"),) * n_outs
        self.sharded = jax.jit(
            shard_map(
                _body, mesh=mesh, in_specs=in_specs, out_specs=out_specs,
                check_rep=False,
            ),
            donate_argnums=donate,
            keep_unused=True,
        )
        self.shard = NamedSharding(mesh, PartitionSpec("core"))
        self.in_names, self.out_names = in_names, out_names
        self.out_avals, self.zero_shapes = out_avals, zero_shapes
        self.n_cores = n_cores

    def prep_inputs(self, in_maps):
        jax = self.jax
        concat = [
            np.concatenate([np.asarray(m[name]) for m in in_maps], axis=0)
            for name in self.in_names
        ]
        dev = [jax.device_put(a, self.shard) for a in concat]
        jax.block_until_ready(dev)
        return dev

    def fresh_zeros(self):
        jax = self.jax
        zs = [
            jax.device_put(
                np.zeros((self.n_cores * s[0], *s[1:]), d), self.shard
            )
            for (s, d) in self.zero_shapes
        ]
        jax.block_until_ready(zs)
        return zs

    def run(self, dev_in, zs):
        out = self.sharded(*dev_in, *zs)
        self.jax.block_until_ready(out)
        return out

    def run_maps(self, in_maps):
        out = self.run(self.prep_inputs(in_maps), self.fresh_zeros())
        return [
            {
                name: np.asarray(out[i]).reshape(
                    self.n_cores, *self.out_avals[i].shape
                )[c]
                for i, name in enumerate(self.out_names)
            }
            for c in range(self.n_cores)
        ]


def _get_runner():
    if "r" not in _CACHE:
        _CACHE["r"] = _Runner(_build_module())
    return _CACHE["r"]


def _kernel_np(x):
    """Exact reference semantics (including jax's clamp of the f32 mod
    boundary case) — robustness fallback if the device executor fails."""
    H, W = _H, _W
    img = x[..., 0:3]
    dx = x[..., 3]
    dy = x[..., 4]
    cols = np.arange(W, dtype=np.float32)
    rows = np.arange(H, dtype=np.float32)[:, None]
    Xi = np.minimum(
        np.mod(cols[None, None, :] + dx, np.float32(W)).astype(np.int32), W - 1
    )
    Yi = np.minimum(
        np.mod(rows[None, :, :] + dy, np.float32(H)).astype(np.int32), H - 1
    )
    b = np.arange(x.shape[0])[:, None, None]
    return img[b, Yi, Xi]


def _kernel_jax_device(x):
    """Tier-2 fallback: run the warp gather on the 8 NeuronCores via
    XLA-Neuron's native gather path."""
    import jax
    import jax.numpy as jnp

    H, W = _H, _W

    def body(xs):  # [B, H, W, 5] per device
        img = xs[..., 0:3]
        dx = xs[..., 3]
        dy = xs[..., 4]
        cols = jnp.arange(W, dtype=jnp.float32)
        rows = jnp.arange(H, dtype=jnp.float32)[:, None]
        Xi = jnp.mod(cols[None, None, :] + dx, float(W)).astype(jnp.int32)
        Yi = jnp.mod(rows[None, :, :] + dy, float(H)).astype(jnp.int32)
        b = jnp.arange(xs.shape[0])[:, None, None]
        return img[b, Yi, Xi]

    if "jdk" not in _CACHE:
        _CACHE["jdk"] = jax.jit(body)
    f = _CACHE["jdk"]
    devices = jax.devices()[:8]
    shards = x.reshape(8, _B, H, W, 5)
    dev_in = [jax.device_put(shards[i], devices[i]) for i in range(8)]
    outs = [f(s) for s in dev_in]
    host = jax.device_get(outs)
    return np.concatenate(host, axis=0)


_USE_BASS = True


def kernel(x):
    x = np.ascontiguousarray(np.asarray(x, dtype=np.float32))
    assert x.shape == (128, _H, _W, 5), x.shape
    n_cores = 8
    # The Bass kernel assumes displacements fit the 13x13 window.
    small = (
        np.abs(x[..., 3]).max() <= float(_HALO)
        and np.abs(x[..., 4]).max() <= float(_HALO)
    )
    if _USE_BASS and small:
        try:
            shards = x.reshape(n_cores, _B * _H, _W * 5)
            in_maps = [{"x": shards[c]} for c in range(n_cores)]
            outs = _get_runner().run_maps(in_maps)
            y = np.stack([o["y"] for o in outs])  # [8, B*H, W*3]
            return y.reshape(128, _H, _W, 3)
        except Exception as e:
            sys.stderr.write(
                f"kernel: bass path failed ({e!r}); jax-device fallback\n"
            )
    try:
        return _kernel_jax_device(x)
    except Exception as e:
        sys.stderr.write(f"kernel: jax-device failed ({e!r}); numpy fallback\n")
        return _kernel_np(x)


# revision 14
# speedup vs baseline: 1.5274x; 1.5274x over previous
"""Trainium2 Bass kernel for nn_Bilinear_70222715290053.

Problem: x [128, 224, 224, 5] f32 where channels 0:3 are an image and
channels 3,4 are per-pixel displacements (dx, dy). Output [128,224,224,3]:
  out[b,i,j,:] = img[b, int(mod(i+dy, 224)), int(mod(j+dx, 224)), :]

Key property: dx, dy ~ N(0,1), so |displacement| <= ~5.5 — the gather is a
LOCAL warp within a 13x13 window (verified on the fixed inputs; kernel()
checks the bound at runtime and falls back if violated).

Strategy (pure data parallel, batch sharded 8 ways — 16 images/core):
  - Rows live in the PARTITION dim: a round processes G=4 images x 112
    output rows on 124 partitions (112 rows + 6 halo rows each side, with
    mod-224 wrap rows loaded into the halo slots). A window shift (oy, ox)
    is then just a partition-offset + free-offset AP — no data duplication.
  - Planarize the 3 image channels into x-wrap-padded planes [124, G*236].
  - Compute source indices exactly in f32 (bit-exact vs the CPU jax
    reference: wrap via compare+add/sub, floor via round-nearest convert +
    is_gt fixup, clamp at 223), normalize to window offsets, fold into a
    single code = offy*13 + offx (fp16).
  - 169 select terms: mask = is_equal(code, t) [fp16, DVE 2x mode], then 3
    copy_predicated moves from the (oy, ox)-shifted plane APs into the
    interleaved output tile. Every pixel matches exactly one term.
  - DMA the interleaved [112, G*672] tile to the output.

Self-contained: builds the Bass module, compiles through neuronx_cc via the
bass2jax custom call, and runs SPMD on 8 NeuronCores via shard_map.
"""

import sys

sys.path.insert(0, "/opt/trn_rl_repo")

import numpy as np

_CACHE = {}

_B, _H, _W = 16, 224, 224  # per-core shard
_HALO = 6
_G = 4  # images per round


def _build_module(B=_B, H=_H, W=_W, G=_G, HALO=_HALO):
    from concourse import mybir, bacc
    import concourse.tile as tile

    F32 = mybir.dt.float32
    F16 = mybir.dt.float16
    I32 = mybir.dt.int32
    I16 = mybir.dt.int16
    Alu = mybir.AluOpType

    NB = 2               # row blocks per image
    RB = H // NB         # 112 output rows per block
    WPAD = W + 2 * HALO  # 236 x-padded plane width
    R5 = W * 5
    R3 = W * 3
    # Band partition layout: 0..RB-1 = central rows r0..r0+RB-1,
    # RB..RB+HALO-1 = top halo rows r0-HALO..r0-1 (mod H),
    # RB+HALO..RB+2*HALO-1 = bottom halo rows r0+RB..r0+RB+HALO-1 (mod H).
    PT = RB            # top halo partition base (112)
    PB = RB + HALO     # bottom halo partition base (118)
    BAND = RB + 2 * HALO

    nc = bacc.Bacc(None, target_bir_lowering=False)
    x = nc.declare_dram_parameter("x", [B * H, R5], F32, isOutput=False)
    y = nc.declare_dram_parameter("y", [B * H, R3], F32, isOutput=True)

    with tile.TileContext(nc) as tc:
        with (
            tc.tile_pool(name="consts", bufs=1) as cpool,
            tc.tile_pool(name="rec", bufs=2) as rpool,
            tc.tile_pool(name="planes", bufs=2) as ppool,
            tc.tile_pool(name="shift", bufs=2) as spool,
            tc.tile_pool(name="tmp", bufs=1) as tpool,
            tc.tile_pool(name="outp", bufs=2) as opool,
        ):
            jpat = cpool.tile([128, G * W], F32, tag="jpat")
            nc.gpsimd.iota(
                jpat[:], pattern=[[0, G], [1, W]], base=0, channel_multiplier=0,
                allow_small_or_imprecise_dtypes=True,
            )
            # absolute row index per partition, one tile per row block, so
            # i + dy is a SINGLE f32 add (bit-exact vs the reference; a
            # chained (dy + q) + r0 double-rounds at integer boundaries)
            rowt = []
            for blk in range(2):
                rt = cpool.tile([128, 1], F32, tag=f"rowt{blk}")
                nc.gpsimd.iota(
                    rt[:], pattern=[[0, 1]], base=blk * (H // 2),
                    channel_multiplier=1,
                    allow_small_or_imprecise_dtypes=True,
                )
                rowt.append(rt)

            for g0 in range(0, B, G):
                for blk in range(NB):
                    r0 = blk * RB
                    rtop = (r0 - HALO) % H    # first top-halo row
                    rbot = (r0 + RB) % H      # first bottom-halo row
                    rec = rpool.tile([128, G * R5], F32, tag="rec")
                    for gi in range(G):
                        b = g0 + gi
                        fs = slice(gi * R5, (gi + 1) * R5)
                        nc.sync.dma_start(
                            out=rec[0:RB, fs],
                            in_=x[b * H + r0 : b * H + r0 + RB, :],
                        )
                        nc.sync.dma_start(
                            out=rec[PT : PT + HALO, fs],
                            in_=x[b * H + rtop : b * H + rtop + HALO, :],
                        )
                        nc.sync.dma_start(
                            out=rec[PB : PB + HALO, fs],
                            in_=x[b * H + rbot : b * H + rbot + HALO, :],
                        )

                    rec4 = rec[:].rearrange("p (g w c) -> p g w c", g=G, c=5)

                    # x-wrap-padded channel planes [BAND, G, WPAD] (band order)
                    planes = []
                    for c in range(3):
                        pc = ppool.tile([128, G * WPAD], F32, tag=f"plane{c}")
                        pc3 = pc[:].rearrange("p (g w) -> p g w", g=G)
                        nc.vector.tensor_copy(
                            out=pc3[0:BAND, :, HALO : HALO + W],
                            in_=rec4[0:BAND, :, :, c : c + 1].rearrange(
                                "p g w k -> p g (w k)"
                            ),
                        )
                        nc.vector.tensor_copy(
                            out=pc3[0:BAND, :, 0:HALO],
                            in_=rec4[0:BAND, :, W - HALO : W, c : c + 1].rearrange(
                                "p g w k -> p g (w k)"
                            ),
                        )
                        nc.vector.tensor_copy(
                            out=pc3[0:BAND, :, HALO + W : WPAD],
                            in_=rec4[0:BAND, :, 0:HALO, c : c + 1].rearrange(
                                "p g w k -> p g (w k)"
                            ),
                        )
                        planes.append(pc)

                    dxv = rec4[0:RB, :, :, 3:4].rearrange("p g w k -> p g (w k)")
                    dyv = rec4[0:RB, :, :, 4:5].rearrange("p g w k -> p g (w k)")

                    A = tpool.tile([128, G * W], F32, tag="A")
                    M = tpool.tile([128, G * W], F32, tag="M")
                    XI = tpool.tile([128, G * W], I32, tag="XI")
                    OX = tpool.tile([128, G * W], F32, tag="OX")
                    OY = tpool.tile([128, G * W], F32, tag="OY")

                    def s(t):
                        return t[0:RB, :]

                    def s3(t):
                        return t[0:RB, :].rearrange("p (g w) -> p g w", g=G)

                    def floor_clamp_off(t, out_off, pat_is_j):
                        """out_off = wrapnorm(min(floor(t), 223) - base)."""
                        # floor via round-nearest int convert + is_gt fixup
                        nc.vector.tensor_copy(out=s(XI), in_=t)
                        nc.vector.tensor_copy(out=out_off, in_=s(XI))
                        nc.vector.tensor_tensor(
                            out=s(M), in0=out_off, in1=t, op=Alu.is_gt
                        )
                        nc.vector.tensor_tensor(
                            out=out_off, in0=out_off, in1=s(M), op=Alu.subtract
                        )
                        nc.vector.tensor_scalar(
                            out=out_off, in0=out_off,
                            scalar1=float(W - 1), scalar2=0.0,
                            op0=Alu.min, op1=Alu.max,
                        )
                        # subtract base index (j or i)
                        if pat_is_j:
                            nc.vector.tensor_tensor(
                                out=out_off, in0=out_off,
                                in1=jpat[0:RB, :],
                                op=Alu.subtract,
                            )
                        else:
                            nc.vector.tensor_scalar(
                                out=out_off, in0=out_off,
                                scalar1=rowt[blk][0:RB, 0:1], scalar2=None,
                                op0=Alu.subtract,
                            )
                        # normalize wrap offsets into [-HALO, HALO]
                        nc.vector.tensor_scalar(
                            out=s(M), in0=out_off, scalar1=112.0, scalar2=None,
                            op0=Alu.is_gt,
                        )
                        nc.vector.scalar_tensor_tensor(
                            out=out_off, in0=s(M), scalar=-224.0, in1=out_off,
                            op0=Alu.mult, op1=Alu.add,
                        )
                        nc.vector.tensor_scalar(
                            out=s(M), in0=out_off, scalar1=-112.0, scalar2=None,
                            op0=Alu.is_lt,
                        )
                        nc.vector.scalar_tensor_tensor(
                            out=out_off, in0=s(M), scalar=224.0, in1=out_off,
                            op0=Alu.mult, op1=Alu.add,
                        )

                    # X: tX = j + dx, wrap into [0, 224], floor, clamp, - j
                    nc.vector.tensor_tensor(
                        out=s(A), in0=dxv, in1=jpat[0:RB, :], op=Alu.add,
                    )
                    nc.vector.tensor_scalar(
                        out=s(M), in0=s(A), scalar1=float(W), scalar2=None,
                        op0=Alu.is_ge,
                    )
                    nc.vector.scalar_tensor_tensor(
                        out=s(A), in0=s(M), scalar=float(-W), in1=s(A),
                        op0=Alu.mult, op1=Alu.add,
                    )
                    nc.vector.tensor_scalar(
                        out=s(M), in0=s(A), scalar1=0.0, scalar2=None,
                        op0=Alu.is_lt,
                    )
                    nc.vector.scalar_tensor_tensor(
                        out=s(A), in0=s(M), scalar=float(W), in1=s(A),
                        op0=Alu.mult, op1=Alu.add,
                    )
                    floor_clamp_off(s(A), s(OX), pat_is_j=True)

                    # Y: tY = i + dy (single add), wrap, floor, clamp, - i
                    nc.vector.tensor_scalar(
                        out=s(A), in0=dyv, scalar1=rowt[blk][0:RB, 0:1],
                        scalar2=None, op0=Alu.add,
                    )
                    if blk == NB - 1:
                        nc.vector.tensor_scalar(
                            out=s(M), in0=s(A), scalar1=float(H), scalar2=None,
                            op0=Alu.is_ge,
                        )
                        nc.vector.scalar_tensor_tensor(
                            out=s(A), in0=s(M), scalar=float(-H), in1=s(A),
                            op0=Alu.mult, op1=Alu.add,
                        )
                    if blk == 0:
                        nc.vector.tensor_scalar(
                            out=s(M), in0=s(A), scalar1=0.0, scalar2=None,
                            op0=Alu.is_lt,
                        )
                        nc.vector.scalar_tensor_tensor(
                            out=s(A), in0=s(M), scalar=float(H), in1=s(A),
                            op0=Alu.mult, op1=Alu.add,
                        )
                    floor_clamp_off(s(A), s(OY), pat_is_j=False)

                    # code = offy*13 + offx in [-84, 84]
                    nc.vector.scalar_tensor_tensor(
                        out=s(A), in0=s(OY), scalar=13.0, in1=s(OX),
                        op0=Alu.mult, op1=Alu.add,
                    )
                    codeh = tpool.tile([128, G * W], F16, tag="codeh")
                    nc.vector.tensor_copy(out=s(codeh), in_=s(A))

                    # Pad per-image strides of mask/out so their interp views
                    # cannot dim-merge (copy_predicated needs all three
                    # operand views shaped identically (RB, G, W)).
                    OSTR = R3 + 16  # 688
                    MSTR = W + 16   # 240
                    out_t = opool.tile([128, G * OSTR], F32, tag="out")
                    out4 = (
                        out_t[:]
                        .rearrange("p (g q) -> p g q", g=G)[:, :, 0:R3]
                        .rearrange("p g (w c) -> p g w c", c=3)
                    )
                    maskh = tpool.tile([128, G * MSTR], I16, tag="maskh")
                    mview = maskh[:].rearrange("p (g q) -> p g q", g=G)[
                        0:RB, :, 0:W
                    ]

                    for oy in range(-HALO, HALO + 1):
                        # partition-shifted copies (DMA may start at any
                        # partition; compute engines may not). Band layout
                        # makes each shift at most 2 contiguous pieces.
                        if oy == 0:
                            sps = planes
                        else:
                            sps = []
                            for c in range(3):
                                sp = spool.tile(
                                    [128, G * WPAD], F32, tag=f"sp{c}"
                                )
                                if oy < 0:
                                    k = -oy
                                    nc.sync.dma_start(
                                        out=sp[0:k, :],
                                        in_=planes[c][PB - k : PB, :],
                                    )
                                    nc.sync.dma_start(
                                        out=sp[k:RB, :],
                                        in_=planes[c][0 : RB - k, :],
                                    )
                                else:
                                    nc.sync.dma_start(
                                        out=sp[0 : RB - oy, :],
                                        in_=planes[c][oy:RB, :],
                                    )
                                    nc.sync.dma_start(
                                        out=sp[RB - oy : RB, :],
                                        in_=planes[c][PB : PB + oy, :],
                                    )
                                sps.append(sp)
                        spv = [
                            t[:].rearrange("p (g w) -> p g w", g=G) for t in sps
                        ]
                        for ox in range(-HALO, HALO + 1):
                            nc.vector.tensor_scalar(
                                out=mview, in0=s3(codeh),
                                scalar1=float(oy * 13 + ox), scalar2=None,
                                op0=Alu.is_equal,
                            )
                            for c in range(3):
                                nc.vector.copy_predicated(
                                    out4[0:RB, :, :, c : c + 1].rearrange(
                                        "p g w k -> p g (w k)"
                                    ),
                                    mview,
                                    spv[c][0:RB, :, HALO + ox : HALO + ox + W],
                                )

                    for gi in range(G):
                        b = g0 + gi
                        nc.sync.dma_start(
                            out=y[b * H + r0 : b * H + r0 + RB, :],
                            in_=out_t[0:RB, gi * OSTR : gi * OSTR + R3],
                        )
    return nc


def _split_multiwait_drains(nc):
    """This walrus build accepts one sync wait per Drain (TPB_CTRL); split
    the Tile epilogue's multi-wait drains into single-wait chains."""
    import copy
    import bass_rust
    from concourse import mybir

    changed = False
    new_functions = []
    for function in nc.m.functions:
        new_function = copy.replace(function, blocks=[])
        new_function.set_allocations_from_list(function.allocations)
        for block in function.blocks:
            new_insts = []
            for ins in block.instructions:
                si = ins.sync_info
                if (
                    isinstance(ins, (mybir.InstDrain, mybir.InstNoOp))
                    and si is not None
                    and len(si.on_wait) > 1
                ):
                    changed = True
                    waits = list(si.on_wait)
                    for i, w in enumerate(waits[:-1]):
                        d = mybir.InstDrain(
                            name=f"{ins.name}_sw{i}", ins=[], outs=[],
                            bass_is_fusable=False,
                        )
                        d.engine = ins.engine
                        d.sync_info = bass_rust.SyncInfo(on_wait=[w], on_update=[])
                        new_insts.append(d)
                    ins.sync_info = bass_rust.SyncInfo(
                        on_wait=[waits[-1]], on_update=list(si.on_update)
                    )
                new_insts.append(ins)
            new_function.blocks.append(copy.replace(block, instructions=new_insts))
        new_functions.append(new_function)
    if changed:
        nc.m = copy.replace(nc.m, functions=new_functions)
    return nc


class _Runner:
    def __init__(self, nc, n_cores=8):
        import jax
        from jax.sharding import Mesh, PartitionSpec, NamedSharding
        from jax.experimental.shard_map import shard_map
        from concourse import mybir
        from concourse.bass2jax import (
            _bass_exec_p,
            install_neuronx_cc_hook,
            partition_id_tensor,
        )

        install_neuronx_cc_hook()
        if not nc.is_finalized():
            nc.finalize()
        _split_multiwait_drains(nc)

        self.jax = jax
        partition_name = (
            nc.partition_id_tensor.name if nc.partition_id_tensor else None
        )
        in_names, out_names, out_avals, zero_shapes = [], [], [], []
        for alloc in nc.m.functions[0].allocations:
            if not isinstance(alloc, mybir.MemoryLocationSet):
                continue
            name = alloc.memorylocations[0].name
            if alloc.kind == "ExternalInput":
                if name != partition_name:
                    in_names.append(name)
            elif alloc.kind == "ExternalOutput":
                out_names.append(name)
                shape = tuple(alloc.tensor_shape)
                dtype = mybir.dt.np(alloc.dtype)
                out_avals.append(jax.core.ShapedArray(shape, dtype))
                zero_shapes.append((shape, dtype))
        n_params = len(in_names)
        n_outs = len(out_avals)
        all_in_names = list(in_names) + list(out_names)
        if partition_name is not None:
            all_in_names.append(partition_name)
        donate = tuple(range(n_params, n_params + n_outs))

        def _body(*args):
            operands = list(args)
            if partition_name is not None:
                operands.append(partition_id_tensor())
            outs = _bass_exec_p.bind(
                *operands,
                out_avals=tuple(out_avals),
                in_names=tuple(all_in_names),
                out_names=tuple(out_names),
                lowering_input_output_aliases=(),
                sim_require_finite=True,
                sim_require_nnan=True,
                nc=nc,
            )
            return tuple(outs)

        devices = jax.devices()[:n_cores]
        mesh = Mesh(np.asarray(devices), ("core",))
        in_specs = (PartitionSpec("core"),) * (n_params + n_outs)
        out_specs = (PartitionSpec("core"),) * n_outs
        self.sharded = jax.jit(
            shard_map(
                _body, mesh=mesh, in_specs=in_specs, out_specs=out_specs,
                check_rep=False,
            ),
            donate_argnums=donate,
            keep_unused=True,
        )
        self.shard = NamedSharding(mesh, PartitionSpec("core"))
        self.in_names, self.out_names = in_names, out_names
        self.out_avals, self.zero_shapes = out_avals, zero_shapes
        self.n_cores = n_cores

    def prep_inputs(self, in_maps):
        jax = self.jax
        concat = [
            np.concatenate([np.asarray(m[name]) for m in in_maps], axis=0)
            for name in self.in_names
        ]
        dev = [jax.device_put(a, self.shard) for a in concat]
        jax.block_until_ready(dev)
        return dev

    def fresh_zeros(self):
        jax = self.jax
        zs = [
            jax.device_put(
                np.zeros((self.n_cores * s[0], *s[1:]), d), self.shard
            )
            for (s, d) in self.zero_shapes
        ]
        jax.block_until_ready(zs)
        return zs

    def run(self, dev_in, zs):
        out = self.sharded(*dev_in, *zs)
        self.jax.block_until_ready(out)
        return out

    def run_maps(self, in_maps):
        out = self.run(self.prep_inputs(in_maps), self.fresh_zeros())
        return [
            {
                name: np.asarray(out[i]).reshape(
                    self.n_cores, *self.out_avals[i].shape
                )[c]
                for i, name in enumerate(self.out_names)
            }
            for c in range(self.n_cores)
        ]


def _get_runner():
    if "r" not in _CACHE:
        _CACHE["r"] = _Runner(_build_module())
    return _CACHE["r"]


def _kernel_np(x):
    """Exact reference semantics (including jax's clamp of the f32 mod
    boundary case) — robustness fallback if the device executor fails."""
    H, W = _H, _W
    img = x[..., 0:3]
    dx = x[..., 3]
    dy = x[..., 4]
    cols = np.arange(W, dtype=np.float32)
    rows = np.arange(H, dtype=np.float32)[:, None]
    Xi = np.minimum(
        np.mod(cols[None, None, :] + dx, np.float32(W)).astype(np.int32), W - 1
    )
    Yi = np.minimum(
        np.mod(rows[None, :, :] + dy, np.float32(H)).astype(np.int32), H - 1
    )
    b = np.arange(x.shape[0])[:, None, None]
    return img[b, Yi, Xi]


def _kernel_jax_device(x):
    """Tier-2 fallback: run the warp gather on the 8 NeuronCores via
    XLA-Neuron's native gather path."""
    import jax
    import jax.numpy as jnp

    H, W = _H, _W

    def body(xs):  # [B, H, W, 5] per device
        img = xs[..., 0:3]
        dx = xs[..., 3]
        dy = xs[..., 4]
        cols = jnp.arange(W, dtype=jnp.float32)
        rows = jnp.arange(H, dtype=jnp.float32)[:, None]
        Xi = jnp.mod(cols[None, None, :] + dx, float(W)).astype(jnp.int32)
        Yi = jnp.mod(rows[None, :, :] + dy, float(H)).astype(jnp.int32)
        b = jnp.arange(xs.shape[0])[:, None, None]
        return img[b, Yi, Xi]

    if "jdk" not in _CACHE:
        _CACHE["jdk"] = jax.jit(body)
    f = _CACHE["jdk"]
    devices = jax.devices()[:8]
    shards = x.reshape(8, _B, H, W, 5)
    dev_in = [jax.device_put(shards[i], devices[i]) for i in range(8)]
    outs = [f(s) for s in dev_in]
    host = jax.device_get(outs)
    return np.concatenate(host, axis=0)


_USE_BASS = True


def kernel(x):
    x = np.ascontiguousarray(np.asarray(x, dtype=np.float32))
    assert x.shape == (128, _H, _W, 5), x.shape
    n_cores = 8
    # The Bass kernel assumes displacements fit the 13x13 window.
    small = (
        np.abs(x[..., 3]).max() <= float(_HALO)
        and np.abs(x[..., 4]).max() <= float(_HALO)
    )
    if _USE_BASS and small:
        try:
            shards = x.reshape(n_cores, _B * _H, _W * 5)
            in_maps = [{"x": shards[c]} for c in range(n_cores)]
            outs = _get_runner().run_maps(in_maps)
            y = np.stack([o["y"] for o in outs])  # [8, B*H, W*3]
            return y.reshape(128, _H, _W, 3)
        except Exception as e:
            sys.stderr.write(
                f"kernel: bass path failed ({e!r}); jax-device fallback\n"
            )
    try:
        return _kernel_jax_device(x)
    except Exception as e:
        sys.stderr.write(f"kernel: jax-device failed ({e!r}); numpy fallback\n")
        return _kernel_np(x)


# revision 16
# speedup vs baseline: 109.0753x; 71.4109x over previous
"""Trainium2 Bass kernel for nn_Bilinear_70222715290053.

Problem: x [128, 224, 224, 5] f32 where channels 0:3 are an image and
channels 3,4 are per-pixel displacements (dx, dy). Output [128,224,224,3]:
  out[b,i,j,:] = img[b, int(mod(i+dy, 224)), int(mod(j+dx, 224)), :]

Key property: dx, dy ~ N(0,1), so |displacement| <= ~5.5 — the gather is a
LOCAL warp within a 13x13 window (verified on the fixed inputs; kernel()
checks the bound at runtime and falls back if violated).

Strategy (pure data parallel, batch sharded 8 ways — 16 images/core):
  - Rows live in the PARTITION dim: a round processes G=4 images x 112
    output rows on 124 partitions (112 rows + 6 halo rows each side, with
    mod-224 wrap rows loaded into the halo slots). A window shift (oy, ox)
    is then just a partition-offset + free-offset AP — no data duplication.
  - Planarize the 3 image channels into x-wrap-padded planes [124, G*236].
  - Compute source indices exactly in f32 (bit-exact vs the CPU jax
    reference: wrap via compare+add/sub, floor via round-nearest convert +
    is_gt fixup, clamp at 223), normalize to window offsets, fold into a
    single code = offy*13 + offx (fp16).
  - 169 select terms: mask = is_equal(code, t) [fp16, DVE 2x mode], then 3
    copy_predicated moves from the (oy, ox)-shifted plane APs into the
    interleaved output tile. Every pixel matches exactly one term.
  - DMA the interleaved [112, G*672] tile to the output.

Self-contained: builds the Bass module, compiles through neuronx_cc via the
bass2jax custom call, and runs SPMD on 8 NeuronCores via shard_map.
"""

import sys

sys.path.insert(0, "/opt/trn_rl_repo")

import numpy as np

_CACHE = {}

_B, _H, _W = 16, 224, 224  # per-core shard
_HALO = 6
_G = 4  # images per round


def _build_module(B=_B, H=_H, W=_W, G=_G, HALO=_HALO):
    from concourse import mybir, bacc
    import concourse.tile as tile

    F32 = mybir.dt.float32
    F16 = mybir.dt.float16
    I32 = mybir.dt.int32
    I16 = mybir.dt.int16
    Alu = mybir.AluOpType

    NB = 2               # row blocks per image
    RB = H // NB         # 112 output rows per block
    WPAD = W + 2 * HALO  # 236 x-padded plane width
    R5 = W * 5
    R3 = W * 3
    # Band partition layout: 0..RB-1 = central rows r0..r0+RB-1,
    # RB..RB+HALO-1 = top halo rows r0-HALO..r0-1 (mod H),
    # RB+HALO..RB+2*HALO-1 = bottom halo rows r0+RB..r0+RB+HALO-1 (mod H).
    PT = RB            # top halo partition base (112)
    PB = RB + HALO     # bottom halo partition base (118)
    BAND = RB + 2 * HALO

    nc = bacc.Bacc(None, target_bir_lowering=False)
    x = nc.declare_dram_parameter("x", [B * H, R5], F32, isOutput=False)
    y = nc.declare_dram_parameter("y", [B * H, R3], F32, isOutput=True)

    with tile.TileContext(nc) as tc:
        with (
            tc.tile_pool(name="consts", bufs=1) as cpool,
            tc.tile_pool(name="rec", bufs=2) as rpool,
            tc.tile_pool(name="planes", bufs=2) as ppool,
            tc.tile_pool(name="shift", bufs=2) as spool,
            tc.tile_pool(name="tmp", bufs=1) as tpool,
            tc.tile_pool(name="outp", bufs=2) as opool,
        ):
            jpat = cpool.tile([128, G * W], F32, tag="jpat")
            nc.gpsimd.iota(
                jpat[:], pattern=[[0, G], [1, W]], base=0, channel_multiplier=0,
                allow_small_or_imprecise_dtypes=True,
            )
            # absolute row index per partition, one tile per row block, so
            # i + dy is a SINGLE f32 add (bit-exact vs the reference; a
            # chained (dy + q) + r0 double-rounds at integer boundaries)
            rowt = []
            for blk in range(2):
                rt = cpool.tile([128, 1], F32, tag=f"rowt{blk}")
                nc.gpsimd.iota(
                    rt[:], pattern=[[0, 1]], base=blk * (H // 2),
                    channel_multiplier=1,
                    allow_small_or_imprecise_dtypes=True,
                )
                rowt.append(rt)

            for g0 in range(0, B, G):
                for blk in range(NB):
                    r0 = blk * RB
                    rtop = (r0 - HALO) % H    # first top-halo row
                    rbot = (r0 + RB) % H      # first bottom-halo row
                    rec = rpool.tile([128, G * R5], F32, tag="rec")
                    for gi in range(G):
                        b = g0 + gi
                        fs = slice(gi * R5, (gi + 1) * R5)
                        nc.sync.dma_start(
                            out=rec[0:RB, fs],
                            in_=x[b * H + r0 : b * H + r0 + RB, :],
                        )
                        nc.sync.dma_start(
                            out=rec[PT : PT + HALO, fs],
                            in_=x[b * H + rtop : b * H + rtop + HALO, :],
                        )
                        nc.sync.dma_start(
                            out=rec[PB : PB + HALO, fs],
                            in_=x[b * H + rbot : b * H + rbot + HALO, :],
                        )

                    rec4 = rec[:].rearrange("p (g w c) -> p g w c", g=G, c=5)

                    # x-wrap-padded channel planes [BAND, G, WPAD] (band order)
                    planes = []
                    for c in range(3):
                        pc = ppool.tile([128, G * WPAD], F32, tag=f"plane{c}")
                        pc3 = pc[:].rearrange("p (g w) -> p g w", g=G)
                        nc.vector.tensor_copy(
                            out=pc3[0:BAND, :, HALO : HALO + W],
                            in_=rec4[0:BAND, :, :, c : c + 1].rearrange(
                                "p g w k -> p g (w k)"
                            ),
                        )
                        nc.vector.tensor_copy(
                            out=pc3[0:BAND, :, 0:HALO],
                            in_=rec4[0:BAND, :, W - HALO : W, c : c + 1].rearrange(
                                "p g w k -> p g (w k)"
                            ),
                        )
                        nc.vector.tensor_copy(
                            out=pc3[0:BAND, :, HALO + W : WPAD],
                            in_=rec4[0:BAND, :, 0:HALO, c : c + 1].rearrange(
                                "p g w k -> p g (w k)"
                            ),
                        )
                        planes.append(pc)

                    dxv = rec4[0:RB, :, :, 3:4].rearrange("p g w k -> p g (w k)")
                    dyv = rec4[0:RB, :, :, 4:5].rearrange("p g w k -> p g (w k)")

                    A = tpool.tile([128, G * W], F32, tag="A")
                    M = tpool.tile([128, G * W], F32, tag="M")
                    XI = tpool.tile([128, G * W], I32, tag="XI")
                    OX = tpool.tile([128, G * W], F32, tag="OX")
                    OY = tpool.tile([128, G * W], F32, tag="OY")

                    def s(t):
                        return t[0:RB, :]

                    def s3(t):
                        return t[0:RB, :].rearrange("p (g w) -> p g w", g=G)

                    def floor_clamp_off(t, out_off, pat_is_j):
                        """out_off = wrapnorm(min(floor(t), 223) - base)."""
                        # floor via round-nearest int convert + is_gt fixup
                        nc.vector.tensor_copy(out=s(XI), in_=t)
                        nc.vector.tensor_copy(out=out_off, in_=s(XI))
                        nc.vector.tensor_tensor(
                            out=s(M), in0=out_off, in1=t, op=Alu.is_gt
                        )
                        nc.vector.tensor_tensor(
                            out=out_off, in0=out_off, in1=s(M), op=Alu.subtract
                        )
                        nc.vector.tensor_scalar(
                            out=out_off, in0=out_off,
                            scalar1=float(W - 1), scalar2=0.0,
                            op0=Alu.min, op1=Alu.max,
                        )
                        # subtract base index (j or i)
                        if pat_is_j:
                            nc.vector.tensor_tensor(
                                out=out_off, in0=out_off,
                                in1=jpat[0:RB, :],
                                op=Alu.subtract,
                            )
                        else:
                            nc.vector.tensor_scalar(
                                out=out_off, in0=out_off,
                                scalar1=rowt[blk][0:RB, 0:1], scalar2=None,
                                op0=Alu.subtract,
                            )
                        # normalize wrap offsets into [-HALO, HALO]
                        nc.vector.tensor_scalar(
                            out=s(M), in0=out_off, scalar1=112.0, scalar2=None,
                            op0=Alu.is_gt,
                        )
                        nc.vector.scalar_tensor_tensor(
                            out=out_off, in0=s(M), scalar=-224.0, in1=out_off,
                            op0=Alu.mult, op1=Alu.add,
                        )
                        nc.vector.tensor_scalar(
                            out=s(M), in0=out_off, scalar1=-112.0, scalar2=None,
                            op0=Alu.is_lt,
                        )
                        nc.vector.scalar_tensor_tensor(
                            out=out_off, in0=s(M), scalar=224.0, in1=out_off,
                            op0=Alu.mult, op1=Alu.add,
                        )

                    # X: tX = j + dx, wrap into [0, 224], floor, clamp, - j
                    nc.vector.tensor_tensor(
                        out=s(A), in0=dxv, in1=jpat[0:RB, :], op=Alu.add,
                    )
                    nc.vector.tensor_scalar(
                        out=s(M), in0=s(A), scalar1=float(W), scalar2=None,
                        op0=Alu.is_ge,
                    )
                    nc.vector.scalar_tensor_tensor(
                        out=s(A), in0=s(M), scalar=float(-W), in1=s(A),
                        op0=Alu.mult, op1=Alu.add,
                    )
                    nc.vector.tensor_scalar(
                        out=s(M), in0=s(A), scalar1=0.0, scalar2=None,
                        op0=Alu.is_lt,
                    )
                    nc.vector.scalar_tensor_tensor(
                        out=s(A), in0=s(M), scalar=float(W), in1=s(A),
                        op0=Alu.mult, op1=Alu.add,
                    )
                    floor_clamp_off(s(A), s(OX), pat_is_j=True)

                    # Y: tY = i + dy (single add), wrap, floor, clamp, - i
                    nc.vector.tensor_scalar(
                        out=s(A), in0=dyv, scalar1=rowt[blk][0:RB, 0:1],
                        scalar2=None, op0=Alu.add,
                    )
                    if blk == NB - 1:
                        nc.vector.tensor_scalar(
                            out=s(M), in0=s(A), scalar1=float(H), scalar2=None,
                            op0=Alu.is_ge,
                        )
                        nc.vector.scalar_tensor_tensor(
                            out=s(A), in0=s(M), scalar=float(-H), in1=s(A),
                            op0=Alu.mult, op1=Alu.add,
                        )
                    if blk == 0:
                        nc.vector.tensor_scalar(
                            out=s(M), in0=s(A), scalar1=0.0, scalar2=None,
                            op0=Alu.is_lt,
                        )
                        nc.vector.scalar_tensor_tensor(
                            out=s(A), in0=s(M), scalar=float(H), in1=s(A),
                            op0=Alu.mult, op1=Alu.add,
                        )
                    floor_clamp_off(s(A), s(OY), pat_is_j=False)

                    # code = offy*13 + offx in [-84, 84]
                    nc.vector.scalar_tensor_tensor(
                        out=s(A), in0=s(OY), scalar=13.0, in1=s(OX),
                        op0=Alu.mult, op1=Alu.add,
                    )
                    codeh = tpool.tile([128, G * W], F16, tag="codeh")
                    nc.vector.tensor_copy(out=s(codeh), in_=s(A))

                    # Pad per-image strides of mask/out so their interp views
                    # cannot dim-merge (copy_predicated needs all three
                    # operand views shaped identically (RB, G, W)).
                    OSTR = R3 + 16  # 688
                    MSTR = W + 16   # 240
                    out_t = opool.tile([128, G * OSTR], F32, tag="out")
                    out4 = (
                        out_t[:]
                        .rearrange("p (g q) -> p g q", g=G)[:, :, 0:R3]
                        .rearrange("p g (w c) -> p g w c", c=3)
                    )
                    maskh = tpool.tile([128, G * MSTR], I16, tag="maskh")
                    mview = maskh[:].rearrange("p (g q) -> p g q", g=G)[
                        0:RB, :, 0:W
                    ]

                    for oy in range(-HALO, HALO + 1):
                        # partition-shifted copies (DMA may start at any
                        # partition; compute engines may not). Band layout
                        # makes each shift at most 2 contiguous pieces.
                        if oy == 0:
                            sps = planes
                        else:
                            sps = []
                            for c in range(3):
                                sp = spool.tile(
                                    [128, G * WPAD], F32, tag=f"sp{c}"
                                )
                                if oy < 0:
                                    k = -oy
                                    nc.sync.dma_start(
                                        out=sp[0:k, :],
                                        in_=planes[c][PB - k : PB, :],
                                    )
                                    nc.sync.dma_start(
                                        out=sp[k:RB, :],
                                        in_=planes[c][0 : RB - k, :],
                                    )
                                else:
                                    nc.sync.dma_start(
                                        out=sp[0 : RB - oy, :],
                                        in_=planes[c][oy:RB, :],
                                    )
                                    nc.sync.dma_start(
                                        out=sp[RB - oy : RB, :],
                                        in_=planes[c][PB : PB + oy, :],
                                    )
                                sps.append(sp)
                        spv = [
                            t[:].rearrange("p (g w) -> p g w", g=G) for t in sps
                        ]
                        for ox in range(-HALO, HALO + 1):
                            nc.vector.tensor_scalar(
                                out=mview, in0=s3(codeh),
                                scalar1=float(oy * 13 + ox), scalar2=None,
                                op0=Alu.is_equal,
                            )
                            for c in range(3):
                                nc.vector.copy_predicated(
                                    out4[0:RB, :, :, c : c + 1].rearrange(
                                        "p g w k -> p g (w k)"
                                    ),
                                    mview,
                                    spv[c][0:RB, :, HALO + ox : HALO + ox + W],
                                )

                    for gi in range(G):
                        b = g0 + gi
                        nc.sync.dma_start(
                            out=y[b * H + r0 : b * H + r0 + RB, :],
                            in_=out_t[0:RB, gi * OSTR : gi * OSTR + R3],
                        )
    return nc


def _split_multiwait_drains(nc):
    """This walrus build accepts one sync wait per Drain (TPB_CTRL); split
    the Tile epilogue's multi-wait drains into single-wait chains."""
    import copy
    import bass_rust
    from concourse import mybir

    changed = False
    new_functions = []
    for function in nc.m.functions:
        new_function = copy.replace(function, blocks=[])
        new_function.set_allocations_from_list(function.allocations)
        for block in function.blocks:
            new_insts = []
            for ins in block.instructions:
                si = ins.sync_info
                if (
                    isinstance(ins, (mybir.InstDrain, mybir.InstNoOp))
                    and si is not None
                    and len(si.on_wait) > 1
                ):
                    changed = True
                    waits = list(si.on_wait)
                    for i, w in enumerate(waits[:-1]):
                        d = mybir.InstDrain(
                            name=f"{ins.name}_sw{i}", ins=[], outs=[],
                            bass_is_fusable=False,
                        )
                        d.engine = ins.engine
                        d.sync_info = bass_rust.SyncInfo(on_wait=[w], on_update=[])
                        new_insts.append(d)
                    ins.sync_info = bass_rust.SyncInfo(
                        on_wait=[waits[-1]], on_update=list(si.on_update)
                    )
                new_insts.append(ins)
            new_function.blocks.append(copy.replace(block, instructions=new_insts))
        new_functions.append(new_function)
    if changed:
        nc.m = copy.replace(nc.m, functions=new_functions)
    return nc


class _Runner:
    def __init__(self, nc, n_cores=8):
        import jax
        from jax.sharding import Mesh, PartitionSpec, NamedSharding
        from jax.experimental.shard_map import shard_map
        from concourse import mybir
        from concourse.bass2jax import (
            _bass_exec_p,
            install_neuronx_cc_hook,
            partition_id_tensor,
        )

        install_neuronx_cc_hook()
        if not nc.is_finalized():
            nc.finalize()
        _split_multiwait_drains(nc)

        self.jax = jax
        partition_name = (
            nc.partition_id_tensor.name if nc.partition_id_tensor else None
        )
        in_names, out_names, out_avals, zero_shapes = [], [], [], []
        for alloc in nc.m.functions[0].allocations:
            if not isinstance(alloc, mybir.MemoryLocationSet):
                continue
            name = alloc.memorylocations[0].name
            if alloc.kind == "ExternalInput":
                if name != partition_name:
                    in_names.append(name)
            elif alloc.kind == "ExternalOutput":
                out_names.append(name)
                shape = tuple(alloc.tensor_shape)
                dtype = mybir.dt.np(alloc.dtype)
                out_avals.append(jax.core.ShapedArray(shape, dtype))
                zero_shapes.append((shape, dtype))
        n_params = len(in_names)
        n_outs = len(out_avals)
        all_in_names = list(in_names) + list(out_names)
        if partition_name is not None:
            all_in_names.append(partition_name)
        donate = tuple(range(n_params, n_params + n_outs))

        def _body(*args):
            operands = list(args)
            if partition_name is not None:
                operands.append(partition_id_tensor())
            outs = _bass_exec_p.bind(
                *operands,
                out_avals=tuple(out_avals),
                in_names=tuple(all_in_names),
                out_names=tuple(out_names),
                lowering_input_output_aliases=(),
                sim_require_finite=True,
                sim_require_nnan=True,
                nc=nc,
            )
            return tuple(outs)

        devices = jax.devices()[:n_cores]
        mesh = Mesh(np.asarray(devices), ("core",))
        in_specs = (PartitionSpec("core"),) * (n_params + n_outs)
        out_specs = (PartitionSpec("core"),) * n_outs
        self.sharded = jax.jit(
            shard_map(
                _body, mesh=mesh, in_specs=in_specs, out_specs=out_specs,
                check_rep=False,
            ),
            donate_argnums=donate,
            keep_unused=True,
        )
        self.devices = devices
        self.shard = NamedSharding(mesh, PartitionSpec("core"))
        self.in_names, self.out_names = in_names, out_names
        self.out_avals, self.zero_shapes = out_avals, zero_shapes
        self.n_cores = n_cores
        # y-init buffers: uploaded once, then recycled from the previous
        # call's outputs (the kernel fully overwrites y, so contents are
        # irrelevant; donation consumes them each call)
        self._ybufs = None

    def prep_inputs(self, in_maps):
        """Upload per-core shards in parallel (one device_put per device)."""
        from concurrent.futures import ThreadPoolExecutor

        jax = self.jax
        arrays = []
        for name in self.in_names:
            shards = [np.asarray(m[name]) for m in in_maps]
            full_shape = (
                self.n_cores * shards[0].shape[0],
                *shards[0].shape[1:],
            )
            with ThreadPoolExecutor(self.n_cores) as ex:
                parts = list(
                    ex.map(
                        lambda t: jax.device_put(t[0], t[1]),
                        zip(shards, self.devices),
                    )
                )
            arrays.append(
                jax.make_array_from_single_device_arrays(
                    full_shape, self.shard, parts
                )
            )
        jax.block_until_ready(arrays)
        return arrays

    def _get_ybufs(self):
        if self._ybufs is None:
            jax = self.jax
            zs = [
                jax.device_put(
                    np.zeros((self.n_cores * s[0], *s[1:]), d), self.shard
                )
                for (s, d) in self.zero_shapes
            ]
            jax.block_until_ready(zs)
            self._ybufs = zs
        return self._ybufs

    def run(self, dev_in):
        out = self.sharded(*dev_in, *self._get_ybufs())
        self.jax.block_until_ready(out)
        self._ybufs = list(out)
        return out

    def fetch(self, out):
        """Download the 8 output shards in parallel."""
        from concurrent.futures import ThreadPoolExecutor

        res = []
        for arr in out:
            shards = sorted(
                arr.addressable_shards, key=lambda s: s.index[0].start
            )
            with ThreadPoolExecutor(self.n_cores) as ex:
                parts = list(ex.map(lambda s: np.asarray(s.data), shards))
            res.append(parts)
        return res

    def run_maps(self, in_maps):
        parts = self.fetch(self.run(self.prep_inputs(in_maps)))
        return [
            {name: parts[i][c] for i, name in enumerate(self.out_names)}
            for c in range(self.n_cores)
        ]


def _get_runner():
    if "r" not in _CACHE:
        _CACHE["r"] = _Runner(_build_module())
    return _CACHE["r"]


def _kernel_np(x):
    """Exact reference semantics (including jax's clamp of the f32 mod
    boundary case) — robustness fallback if the device executor fails."""
    H, W = _H, _W
    img = x[..., 0:3]
    dx = x[..., 3]
    dy = x[..., 4]
    cols = np.arange(W, dtype=np.float32)
    rows = np.arange(H, dtype=np.float32)[:, None]
    Xi = np.minimum(
        np.mod(cols[None, None, :] + dx, np.float32(W)).astype(np.int32), W - 1
    )
    Yi = np.minimum(
        np.mod(rows[None, :, :] + dy, np.float32(H)).astype(np.int32), H - 1
    )
    b = np.arange(x.shape[0])[:, None, None]
    return img[b, Yi, Xi]


def _kernel_jax_device(x):
    """Tier-2 fallback: run the warp gather on the 8 NeuronCores via
    XLA-Neuron's native gather path."""
    import jax
    import jax.numpy as jnp

    H, W = _H, _W

    def body(xs):  # [B, H, W, 5] per device
        img = xs[..., 0:3]
        dx = xs[..., 3]
        dy = xs[..., 4]
        cols = jnp.arange(W, dtype=jnp.float32)
        rows = jnp.arange(H, dtype=jnp.float32)[:, None]
        Xi = jnp.mod(cols[None, None, :] + dx, float(W)).astype(jnp.int32)
        Yi = jnp.mod(rows[None, :, :] + dy, float(H)).astype(jnp.int32)
        b = jnp.arange(xs.shape[0])[:, None, None]
        return img[b, Yi, Xi]

    if "jdk" not in _CACHE:
        _CACHE["jdk"] = jax.jit(body)
    f = _CACHE["jdk"]
    devices = jax.devices()[:8]
    shards = x.reshape(8, _B, H, W, 5)
    dev_in = [jax.device_put(shards[i], devices[i]) for i in range(8)]
    outs = [f(s) for s in dev_in]
    host = jax.device_get(outs)
    return np.concatenate(host, axis=0)


_USE_BASS = True


def kernel(x):
    x = np.ascontiguousarray(np.asarray(x, dtype=np.float32))
    assert x.shape == (128, _H, _W, 5), x.shape
    n_cores = 8
    # The Bass kernel assumes displacements fit the 13x13 window.
    small = (
        np.abs(x[..., 3]).max() <= float(_HALO)
        and np.abs(x[..., 4]).max() <= float(_HALO)
    )
    if _USE_BASS and small:
        try:
            shards = x.reshape(n_cores, _B * _H, _W * 5)
            in_maps = [{"x": shards[c]} for c in range(n_cores)]
            outs = _get_runner().run_maps(in_maps)
            y = np.stack([o["y"] for o in outs])  # [8, B*H, W*3]
            return y.reshape(128, _H, _W, 3)
        except Exception as e:
            sys.stderr.write(
                f"kernel: bass path failed ({e!r}); jax-device fallback\n"
            )
    try:
        return _kernel_jax_device(x)
    except Exception as e:
        sys.stderr.write(f"kernel: jax-device failed ({e!r}); numpy fallback\n")
        return _kernel_np(x)
